# revision 37
# baseline (speedup 1.0000x reference)
# Differential multi-head attention (dual softmax + GroupNorm + sigmoid gating)
# for Trainium2, batch-parallel across 8 NeuronCores (one batch row per core).
#
# Per-core math (batch b):
#   q = query @ Wq + bq -> per head: q1, q2, gate (each S x 64)
#   k = key   @ Wk + bk -> per head: k1, k2
#   v = values@ Wv + bv -> per head: v (S x 64)
#   attn = softmax(q1 k1^T / 8) - lam * softmax(q2 k2^T / 8)
#   out  = GroupNorm_{8 groups over d, reduced over (S, heads, d-in-group)}(attn @ v)
#   out  = out * (1 - lambda_init) * sigmoid(gate)
#
# Layout strategy:
#  - Host pre-packs x^T and all weights as bf16 (layout marshalling only; the
#    math runs on device).  q/k projections are d-major (q1/q2 resp. k1/k2 in
#    complementary 64-partition halves of one [128,S] tile); score matmuls are
#    K=64 at PE row-tiles 0/64 (no zero padding needed).
#  - "Flipped" o-matmul: stationary = exp-score slice [128k x 128q], moving =
#    v_aug [128k x 65] -> out is q-major [128q, 65] with the exp row-sum in
#    column 64 (ones column in v_aug).  This halves PE time vs the d-major
#    o-matmul, makes softmax normalization a per-partition scalar multiply,
#    and leaves y in the exact output layout (no final transposes).
#  - GroupNorm stats via ones-column f32r matmuls (Sum y, Sum y^2 over seq on
#    partitions), group-reduced on DVE, rsqrt via sqrt+NR, then the per-column
#    affine A,B is expanded back to [128,512] with tiny K=1/K=8 matmuls.
#  - ACT runs (nearly) only the 128 [128,1024] exp instructions + 8 tanh; all
#    projection epilogues run on DVE/Pool.  PSUM: s double-buffered (4 banks),
#    o manually packed into 3 banks (7+7+2 groups of 65 cols), proj 1 bank.

import numpy as np

B, S, H, D = 8, 1024, 8, 64
DM = H * D  # 512
NJ = S // 128   # key 128-tiles
NQ = S // 128   # query 128-blocks
EPS = 1e-3
INV = 0.125
CNT = float(S * H)  # groupnorm count per group = S * H * (D//H) / ... = 1024*8


def build_nc():
    import concourse.bacc as bacc
    import concourse.tile as tile
    from concourse import mybir

    f32 = mybir.dt.float32
    f32r = mybir.dt.float32r
    bf16 = mybir.dt.bfloat16
    AF = mybir.ActivationFunctionType
    OP = mybir.AluOpType
    AX = mybir.AxisListType

    nc = bacc.Bacc(target_bir_lowering=False)
    xq_d = nc.dram_tensor("xq", [128, 4 * S], bf16, kind="ExternalInput")
    xk_d = nc.dram_tensor("xk", [128, 4 * S], bf16, kind="ExternalInput")
    xv_d = nc.dram_tensor("xv", [128, 4 * S], bf16, kind="ExternalInput")
    wq_d = nc.dram_tensor("wq", [128, 4 * 1024], bf16, kind="ExternalInput")
    wk_d = nc.dram_tensor("wk", [128, 4 * 1024], bf16, kind="ExternalInput")
    wv_d = nc.dram_tensor("wv", [128, 4 * 512], bf16, kind="ExternalInput")
    wg_d = nc.dram_tensor("wg", [128, 4 * 512], bf16, kind="ExternalInput")
    bqk_d = nc.dram_tensor("bqk", [128, 18], f32, kind="ExternalInput")
    rows_d = nc.dram_tensor("rows", [1, 2048], f32, kind="ExternalInput")
    rowsb_d = nc.dram_tensor("rowsb", [1, 1024], bf16, kind="ExternalInput")
    g8_d = nc.dram_tensor("g8", [8, 512], bf16, kind="ExternalInput")
    out_d = nc.dram_tensor("out", [S, DM], bf16, kind="ExternalOutput")

    ts_ = nc.vector.tensor_scalar
    stt = nc.vector.scalar_tensor_tensor
    gts_ = nc.gpsimd.tensor_scalar
    gstt = nc.gpsimd.scalar_tensor_tensor

    # (t, qb) accumulation group -> (o-bank index, column offset); 65 cols per
    # group (64 d + 1 ones-sum), packed 7 + 7 + 2 into three psum banks.
    def o_loc(t, qb):
        g = t * 8 + qb
        if g < 7:
            return 0, 65 * g
        if g < 14:
            return 1, 65 * (g - 7)
        return 2, 65 * (g - 14)

    with tile.TileContext(nc) as tc:
        with tc.tile_pool(name="persist", bufs=1) as pp:
            # ---------- persistent SBUF ----------
            xq_t = pp.tile([128, 4 * S], bf16, tag="xq_t", name="xq_t")
            xk_t = pp.tile([128, 4 * S], bf16, tag="xk_t", name="xk_t")
            xv_t = pp.tile([128, 4 * S], bf16, tag="xv_t", name="xv_t")
            wq_t = pp.tile([128, 4 * 1024], bf16, tag="wq_t", name="wq_t")
            wk_t = pp.tile([128, 4 * 1024], bf16, tag="wk_t", name="wk_t")
            wv_t = pp.tile([128, 4 * 512], bf16, tag="wv_t", name="wv_t")
            wg_t = pp.tile([128, 4 * 512], bf16, tag="wg_t", name="wg_t")
            bqk_t = pp.tile([128, 18], f32, tag="bqk_t", name="bqk_t")
            rows_t = pp.tile([1, 2048], f32, tag="rows_t", name="rows_t")
            rowsb_t = pp.tile([1, 1024], bf16, tag="rowsb_t", name="rowsb_t")
            g8_t = pp.tile([8, 512], bf16, tag="g8_t", name="g8_t")
            onesb = pp.tile([128, 130], bf16, tag="onesb", name="onesb")
            onesf = pp.tile([1, 129], f32, tag="onesf", name="onesf")
            negl_t = bqk_t[:, 16:17]
            ones_c = onesb[:, 0:1]
            ones_r = onesb[0:1, 0:128]
            ones_rf = onesf[:, 0:128]
            one1 = onesf[:, 128:129]
            qp = [pp.tile([128, S], bf16, tag=f"qp{h}", name=f"qp{h}") for h in range(8)]
            kp = [pp.tile([128, S], bf16, tag=f"kp{h}", name=f"kp{h}") for h in range(8)]
            va_all = pp.tile([128, NJ, 8, 65], bf16, tag="va", name="va")
            va = [va_all[:, j] for j in range(NJ)]
            th = [pp.tile([128, 512], bf16, tag=f"th{q}", name=f"th{q}") for q in range(NQ)]
            graw = [pp.tile([128, 512], bf16, tag=f"gr{q}", name=f"gr{q}") for q in range(NQ)]
            yy = [pp.tile([128, 512], bf16, tag=f"yy{q}", name=f"yy{q}") for q in range(NQ)]
            y2b = [pp.tile([128, 512], bf16, tag=f"y2b{q}", name=f"y2b{q}") for q in range(NQ)]
            wrm = pp.tile([128, 512], bf16, tag="wrm", name="wrm")
            mrst = pp.tile([8, 2], f32, tag="mrst", name="mrst")
            mrstb = pp.tile([8, 2], bf16, tag="mrstb", name="mrstb")

            # ---------- DMAs ----------
            # Three DMA queues (SP, ACT, Pool SW-DGE) run in parallel; the
            # startup-critical set (wq/wk heads 0-1, xq both halves, xk n0)
            # is spread across them so the first scores can fire ~15us in.
            xqv = xq_t.rearrange("p (r s) -> p r s", s=S)
            xkv = xk_t.rearrange("p (r s) -> p r s", s=S)
            xvv = xv_t.rearrange("p (r s) -> p r s", s=S)
            xqd = xq_d[:, :].rearrange("p (r s) -> p r s", s=S)
            xkd = xk_d[:, :].rearrange("p (r s) -> p r s", s=S)
            xvd = xv_d[:, :].rearrange("p (r s) -> p r s", s=S)
            wqv = wq_t.rearrange("p (r c) -> p r c", c=1024)
            wkv = wk_t.rearrange("p (r c) -> p r c", c=1024)
            wqd = wq_d[:, :].rearrange("p (r c) -> p r c", c=1024)
            wkd = wk_d[:, :].rearrange("p (r c) -> p r c", c=1024)
            # Pool queue (SW DGE): merged ones/warmup/va memsets first,
            # then small consts, then mid-deadline bulk.  16 input DMAs
            # total across the queues -- the DMA-sem pool holds ~18, and
            # exceeding it serializes transfers cross-queue via sem reuse
            # (and the waiting descriptor blocks its engine queue).
            nc.gpsimd.memset(onesb, 1.0)
            nc.gpsimd.memset(onesf, 1.0)
            nc.gpsimd.memset(wrm, 0.0)
            nc.gpsimd.memset(va_all[:, :, :, 64:65], 1.0)
            nc.gpsimd.dma_start(out=bqk_t, in_=bqk_d[:, :])
            nc.gpsimd.dma_start(out=rows_t, in_=rows_d[:, :])
            nc.gpsimd.dma_start(out=rowsb_t, in_=rowsb_d[:, :])
            nc.gpsimd.dma_start(out=g8_t, in_=g8_d[:, :])
            nc.gpsimd.dma_start(out=wv_t, in_=wv_d[:, :])
            nc.gpsimd.dma_start(out=xvv[:, :, 512:1024], in_=xvd[:, :, 512:1024])
            nc.gpsimd.dma_start(out=wg_t, in_=wg_d[:, :])
            # SP queue: xq n0, xk n0/n1, xv n0 in arrival-critical order.
            nc.sync.dma_start(out=xqv[:, :, 0:512], in_=xqd[:, :, 0:512])
            nc.sync.dma_start(out=xkv[:, :, 0:512], in_=xkd[:, :, 0:512])
            nc.sync.dma_start(out=xkv[:, :, 512:1024], in_=xkd[:, :, 512:1024])
            nc.sync.dma_start(out=xvv[:, :, 0:512], in_=xvd[:, :, 0:512])
            nc.sync.dma_start(out=wqv[:, :, 256:1024], in_=wqd[:, :, 256:1024])
            # ACT queue: heads-01 weight slices + xq n1, then staged bulk.
            nc.scalar.dma_start(out=wqv[:, :, 0:256], in_=wqd[:, :, 0:256])
            nc.scalar.dma_start(out=xqv[:, :, 512:1024], in_=xqd[:, :, 512:1024])
            nc.scalar.dma_start(out=wkv[:, :, 0:256], in_=wkd[:, :, 0:256])
            nc.scalar.dma_start(out=wkv[:, :, 256:1024], in_=wkd[:, :, 256:1024])

            ghl_r = rows_t[:, 0:512]
            bhl_r = rows_t[:, 512:1024]
            gb_r = rowsb_t[:, 0:512]
            vb_r = rowsb_t[:, 512:1024]

            with tc.tile_pool(name="ps_proj", bufs=1, space="PSUM") as ps_proj, \
                 tc.tile_pool(name="ps_s", bufs=2, space="PSUM") as ps_s, \
                 tc.tile_pool(name="ps_o", bufs=1, space="PSUM") as ps_o, \
                 tc.tile_pool(name="expp", bufs=14) as expp, \
                 tc.tile_pool(name="rscp", bufs=2) as rscp:

                # ---------- projection filler units (one PE matmul each) ----
                # Projections for heads >= 2, v, and gate are emitted as
                # single-matmul units interleaved into the attention loop so
                # the PE stream stays dense while the loop is ACT(exp)-paced.
                def qk_units(h, kind, ns=(0, 1)):
                    if kind == "q":
                        w_t, x_t, dst = wq_t, xq_t, qp[h]
                        bcol = bqk_t[:, h:h + 1]
                        wofs = lambda r: 1024 * r + 128 * h
                    else:
                        w_t, x_t, dst = wk_t, xk_t, kp[h]
                        bcol = bqk_t[:, 8 + h:9 + h]
                        wofs = lambda r: 1024 * r + 128 * h
                    st = {}
                    units = []
                    for n in ns:
                        for r in range(4):
                            def f(n=n, r=r):
                                if r == 0:
                                    st[n] = ps_proj.tile(
                                        [128, 512], f32, tag="proj", name="proj")
                                nc.tensor.matmul(
                                    st[n], w_t[:, wofs(r):wofs(r) + 128],
                                    x_t[:, S * r + 512 * n:S * r + 512 * (n + 1)],
                                    start=(r == 0), stop=(r == 3))
                                if r == 3:
                                    ts_(dst[:, 512 * n:512 * (n + 1)], st[n],
                                        bcol, None, OP.add)
                            units.append(f)
                    return units

                def vg_units(q, kind):
                    # v projection (j-tile q) or gate projection (q-block q)
                    st = {}
                    units = []
                    for r in range(4):
                        def f(r=r):
                            x_t = xv_t if kind == "v" else xq_t
                            w_t = wv_t if kind == "v" else wg_t
                            if r == 0:
                                st[0] = ps_proj.tile(
                                    [128, 512], f32, tag="proj", name="proj")
                            nc.tensor.matmul(
                                st[0], x_t[:, S * r + 128 * q:S * r + 128 * (q + 1)],
                                w_t[:, 512 * r:512 * (r + 1)],
                                start=(r == 0), stop=False)
                            if r == 3:
                                if kind == "v":
                                    nc.tensor.matmul(st[0], ones_r[:], vb_r,
                                                     start=False, stop=True)
                                    nc.vector.tensor_copy(
                                        va[q][:, :, 0:64],
                                        st[0].rearrange("p (h d) -> p h d", d=64))
                                else:
                                    nc.tensor.matmul(st[0], ones_r[:], gb_r,
                                                     start=False, stop=True)
                                    nc.vector.tensor_copy(graw[q], st[0])
                        units.append(f)
                    return units

                # ---------- combine: y[:, 64h:64h+64] = o1*r1 - lam*o2*r2 --
                stats_aps = {}

                def combine(h, ot):
                    rsc = rscp.tile([128, 16], f32, tag="rsc", name="rsc")
                    nc.vector.reciprocal(
                        rsc[:, 0:7],
                        ot[0][:, 0:455].rearrange("p (g c) -> p g c", c=65)[:, :, 64])
                    nc.vector.reciprocal(
                        rsc[:, 7:14],
                        ot[1][:, 0:455].rearrange("p (g c) -> p g c", c=65)[:, :, 64])
                    nc.vector.reciprocal(
                        rsc[:, 14:16],
                        ot[2][:, 0:130].rearrange("p (g c) -> p g c", c=65)[:, :, 64])
                    ts_(rsc[:, 8:16], rsc[:, 8:16], negl_t, None, OP.mult)
                    for qb in range(8):
                        b1, c1 = o_loc(0, qb)
                        b2, c2 = o_loc(1, qb)
                        ydst = yy[qb][:, 64 * h:64 * h + 64]
                        ts_(ydst, ot[b1][:, c1:c1 + 64],
                            rsc[:, qb:qb + 1], None, OP.mult)
                        stt(ydst, ot[b2][:, c2:c2 + 64],
                            rsc[:, 8 + qb:9 + qb], ydst, OP.mult, OP.add)
                        # y^2 accumulates per head on the (mostly idle) Pool
                        # engine so the tail needs no ACT squares; head 7 on
                        # DVE so the tail stats don't wait on cross-engine
                        # sems.
                        y2eng = nc.vector if h == 7 else nc.gpsimd
                        y2eng.tensor_mul(
                            y2b[qb][:, 64 * h:64 * h + 64], ydst, ydst)
                        if h == 7:
                            # fuse GroupNorm stats into the last head's
                            # combine: sy/sy2 live in an s-pool tile (the
                            # exp stream is done with it).
                            if qb == 0:
                                stats_aps["t"] = ps_s.tile(
                                    [128, S], f32, tag="s", name="s_stats")
                            st_t = stats_aps["t"]
                            nc.tensor.matmul(st_t[0:1, 0:512], ones_c[:],
                                             yy[qb],
                                             start=(qb == 0), stop=(qb == 7))
                            nc.tensor.matmul(st_t[0:1, 512:1024], ones_c[:],
                                             y2b[qb],
                                             start=(qb == 0), stop=(qb == 7))

                # ---------- emission ----------
                # PE warmup in the DMA window: dummy matmuls into an o-pool
                # bank (idle until ~step 6) so the first real projections run
                # at full p-state.  wrm is memset first on the Pool queue, so
                # these start ~7us in, well before any input data lands.
                wps = ps_o.tile([128, 512], f32, tag="o0", name="warm")
                for i in range(8):
                    nc.tensor.matmul(wps[0:1, :], ones_c[:], wrm,
                                     start=True, stop=True)
                # Inline, ordered by DMA arrival: qp0/qp1 n0 (wq h01 + xq
                # n0), qp0 n1 (xq n1), a few more warmups to keep the PE
                # p-state hot while xk n0 lands, then kp0 n0.  Everything
                # else becomes ordered filler.
                stA = ps_s.tile([128, S], f32, tag="s", name="sA")
                for r in range(4):
                    nc.tensor.matmul(
                        stA[:, 0:512], wq_t[:, 1024 * r:1024 * r + 128],
                        xq_t[:, S * r:S * r + 512],
                        start=(r == 0), stop=(r == 3))
                ts_(qp[0][:, 0:512], stA[:, 0:512], bqk_t[:, 0:1], None, OP.add)
                for r in range(4):
                    nc.tensor.matmul(
                        stA[:, 512:1024], wq_t[:, 1024 * r + 128:1024 * r + 256],
                        xq_t[:, S * r:S * r + 512],
                        start=(r == 0), stop=(r == 3))
                ts_(qp[1][:, 0:512], stA[:, 512:1024], bqk_t[:, 1:2], None,
                    OP.add)
                stB = ps_s.tile([128, S], f32, tag="s", name="sB")
                for r in range(4):
                    nc.tensor.matmul(
                        stB[:, 0:512], wq_t[:, 1024 * r:1024 * r + 128],
                        xq_t[:, S * r + 512:S * r + 1024],
                        start=(r == 0), stop=(r == 3))
                ts_(qp[0][:, 512:1024], stB[:, 0:512], bqk_t[:, 0:1], None,
                    OP.add)
                for i in range(4):
                    nc.tensor.matmul(wps[0:1, :], ones_c[:], wrm,
                                     start=True, stop=True)
                for r in range(4):
                    nc.tensor.matmul(
                        stB[:, 512:1024], wk_t[:, 1024 * r:1024 * r + 128],
                        xk_t[:, S * r:S * r + 512],
                        start=(r == 0), stop=(r == 3))
                ts_(kp[0][:, 0:512], stB[:, 512:1024], bqk_t[:, 8:9], None,
                    OP.add)

                filler = []
                flabels = []

                def fext(units, label):
                    filler.extend(units)
                    flabels.extend([label] * len(units))

                fext(qk_units(1, "k", ns=(0,)), "kp1n0")
                fext(qk_units(0, "k", ns=(1,)), "kp0n1")
                fext(qk_units(1, "q", ns=(1,)), "qp1n1")
                fext(qk_units(1, "k", ns=(1,)), "kp1n1")
                for j in range(NJ):
                    fext(vg_units(j, "v"), f"v{j}")
                for h2 in range(2, 8):
                    fext(qk_units(h2, "q"), f"qp{h2}")
                    fext(qk_units(h2, "k"), f"kp{h2}")
                for q in range(NQ):
                    fext(vg_units(q, "g"), f"g{q}")
                fdone = []

                # software-pipelined attention loop with a 2-step o-lag:
                # everything PE does in step i depends only on exp(i-2), so
                # the PE never waits mid-step and the exp stream stays dense.
                steps = [(h, t, j) for h in range(8) for t in range(2)
                         for j in range(NJ)]
                otiles = {}
                pending = []

                def emit_o(ph, pt, pj, pex):
                    # program-order safety: the v epilogue writing va[pj]
                    # must already be issued, else no hazard dep exists and
                    # the PE reads garbage (race seen on hw).
                    assert fdone.count(f"v{pj}") == 4 or ph > 0 or pt > 0, \
                        f"va[{pj}] written {fdone.count(f'v{pj}')}/4 at o-emit"
                    if pt == 0 and pj == 0:
                        otiles[ph] = [
                            ps_o.tile([128, 512], f32, tag=f"o{i}", name=f"o{i}")
                            for i in range(3)]
                    ot = otiles[ph]
                    for qb in range(NQ):
                        bi, c0 = o_loc(pt, qb)
                        # start only on the FIRST matmul into each bank this
                        # head: start_tensor_calc zeroes the whole bank, so
                        # later groups accumulate onto the zeroed regions.
                        first = (pj == 0) and (pt, qb) in ((0, 0), (0, 7), (1, 6))
                        nc.tensor.matmul(
                            ot[bi][:, c0:c0 + 65],
                            pex[:, 128 * qb:128 * (qb + 1)],
                            va[pj][:, ph, :],
                            start=first, stop=(pj == NJ - 1),
                            skip_group_check=True)
                    if pt == 1 and pj == NJ - 1:
                        combine(ph, ot)

                def filler_target(i):
                    # cumulative filler units to have popped by end of step i.
                    # Front-loaded (3/step) while the PE has no o-work, so
                    # every v-unit's va write is ISSUED (program order) well
                    # before the first o-matmul that reads it (o-emits start
                    # at idx 13); 1/step after, so each proj group spans 4
                    # steps and its DVE epilogue never blocks the next
                    # group's psum-bank reuse.
                    c = 3 * min(i + 1, 20)
                    c += 2 * max(0, min(i + 1, 32) - 20)
                    c += max(0, i + 1 - 32)
                    return c

                fpopped = 0
                for idx, (h, t, j) in enumerate(steps):
                    # o-matmuls of step i-2 first: they are ready the moment
                    # exp(i-1) starts, so their leading stationary-switch
                    # stall hides under exp; scores of step i follow and
                    # complete well before exp(i-1) ends.
                    odepth = 13 if idx < 24 else max(2, 13 - (idx - 23) // 3)
                    while len(pending) >= odepth:
                        emit_o(*pending.pop(0))
                    s_t = ps_s.tile([128, S], f32, tag="s", name="s")
                    for n in range(2):
                        nc.tensor.matmul(
                            s_t[:, 512 * n:512 * (n + 1)],
                            kp[h][64 * t:64 * t + 64, 128 * j:128 * (j + 1)],
                            qp[h][64 * t:64 * t + 64, 512 * n:512 * (n + 1)],
                            start=True, stop=True)
                    ex = expp.tile([128, S], bf16, tag="ex", name="ex")
                    if j == 3:
                        # Schraudolph on DVE: bf16 bits of exp(s/8) ~=
                        # int16(s * 128*log2(e)/8 + (127*128 - 5.58)); offload
                        # every 16th-ish tile from ACT (the loop pacer).
                        # Softmax normalization cancels most of the ~1.5% rms
                        # approximation error across keys.
                        ts_(ex.bitcast(mybir.dt.int16), s_t,
                            128 * 1.4426950408889634 * INV, 16256.0 - 4.75,
                            OP.mult, OP.add)
                    else:
                        nc.scalar.activation(ex, s_t, AF.Exp, scale=INV)
                    pending.append((h, t, j, ex))
                    while filler and fpopped < filler_target(idx):
                        filler.pop(0)()
                        fdone.append(flabels[fpopped])
                        fpopped += 1
                while pending:
                    emit_o(*pending.pop(0))
                while filler:
                    filler.pop(0)()

            # ---------- tail: GroupNorm stats + affine + gate + output ----
            with tc.tile_pool(name="ps_tail", bufs=1, space="PSUM") as ps_t, \
                 tc.tile_pool(name="oqp", bufs=8) as oqp, \
                 tc.tile_pool(name="tsb", bufs=1) as tsb:

                for q in range(NQ):
                    nc.scalar.activation(th[q], graw[q], AF.Tanh, scale=0.5)
                mcol = ps_t.tile([8, 2], f32, tag="mcol", name="mcol")
                rxA = ps_t.tile([1, 512], f32, tag="rxA", name="rxA")
                rxB = ps_t.tile([1, 512], f32, tag="rxB", name="rxB")
                ab = ps_t.tile([128, 1024], f32, tag="ab", name="ab")

                sy = stats_aps["t"][0:1, 0:512]
                sy2 = stats_aps["t"][0:1, 512:1024]
                gsum = tsb.tile([1, 8], f32, tag="gsum", name="gsum")
                g2 = tsb.tile([1, 8], f32, tag="g2", name="g2")
                nc.vector.tensor_reduce(
                    gsum, sy.rearrange("o (h g e) -> o g h e", h=8, g=8),
                    axis=AX.XY, op=OP.add)
                nc.vector.tensor_reduce(
                    g2, sy2.rearrange("o (h g e) -> o g h e", h=8, g=8),
                    axis=AX.XY, op=OP.add)
                nc.tensor.matmul(mcol[:, 0:1], gsum, one1, start=True, stop=False,
                                 skip_group_check=True)
                nc.tensor.matmul(mcol[:, 1:2], g2, one1, start=False, stop=True,
                                 skip_group_check=True)

                e2t = tsb.tile([8, 1], f32, tag="e2t", name="e2t")
                nm = tsb.tile([8, 1], f32, tag="nm", name="nm")
                veps = tsb.tile([8, 1], f32, tag="veps", name="veps")
                rr = tsb.tile([8, 1], f32, tag="rr", name="rr")
                rsd = tsb.tile([8, 1], f32, tag="rsd", name="rsd")
                ts_(mrst[:, 1:2], mcol[:, 0:1], 1.0 / float(S * H * 8), None, OP.mult)
                ts_(e2t, mcol[:, 1:2], 1.0 / float(S * H * 8), None, OP.mult)
                ts_(nm, mrst[:, 1:2], mrst[:, 1:2], -1.0, OP.mult, OP.mult)
                stt(veps, nm, EPS, e2t, OP.add, OP.add)
                # rsqrt via bit-trick seed + 2 Newton iterations (DVE only,
                # avoids the ACT sqrt table load)
                vi = veps.bitcast(mybir.dt.int32)
                si = rsd.bitcast(mybir.dt.int32)
                ts_(si, vi, 1, None, OP.logical_shift_right)
                ts_(si, si, -1, None, OP.bitwise_xor)
                ts_(si, si, 0x5F3759E0, None, OP.add)
                for _ in range(2):
                    nc.vector.tensor_mul(rr, rsd, rsd)
                    nc.vector.tensor_mul(rr, rr, veps)
                    ts_(rr, rr, -0.5, 1.5, OP.mult, OP.add)
                    nc.vector.tensor_mul(rsd, rsd, rr)
                nc.vector.tensor_copy(mrst[:, 0:1], rsd)

                nc.vector.tensor_copy(mrstb, mrst)
                nc.tensor.matmul(rxA, mrstb[:, 0:1], g8_t[:, :], start=True, stop=True)
                nc.tensor.matmul(rxB, mrstb[:, 1:2], g8_t[:, :], start=True, stop=True)
                arow = tsb.tile([1, 512], bf16, tag="arow", name="arow")
                btmp = tsb.tile([1, 512], f32, tag="btmp", name="btmp")
                brow = tsb.tile([1, 512], bf16, tag="brow", name="brow")
                nc.vector.tensor_mul(arow, ghl_r, rxA)
                nc.vector.tensor_mul(btmp, rxB, arow)
                nc.vector.tensor_sub(brow, bhl_r, btmp)
                nc.tensor.matmul(ab[:, 0:512], ones_r[:], arow,
                                 start=True, stop=True)
                nc.tensor.matmul(ab[:, 512:1024], ones_r[:], brow,
                                 start=True, stop=True)

                ab_sb = tsb.tile([128, 1024], bf16, tag="ab_sb", name="ab_sb")
                nc.vector.tensor_copy(ab_sb[:, 0:512], ab[:, 0:512])
                nc.vector.tensor_copy(ab_sb[:, 512:1024], ab[:, 512:1024])
                # finalize on DVE only (cross-engine DVE/Pool splitting was
                # measured ~2x slower per op: SBUF port contention + sem
                # ping-pong); output DMA round-robins all three queues.
                dmas = (nc.sync.dma_start, nc.scalar.dma_start,
                        nc.gpsimd.dma_start)
                for qb in range(NQ):
                    oq = oqp.tile([128, 512], bf16, tag="oq", name="oq")
                    nc.vector.tensor_mul(oq, yy[qb], ab_sb[:, 0:512])
                    nc.vector.tensor_add(oq, oq, ab_sb[:, 512:1024])
                    stt(oq, th[qb], 1.0, oq, OP.add, OP.mult)
                    dmas[qb % 3](out=out_d[128 * qb:128 * (qb + 1), :], in_=oq)

    nc.finalize()
    return nc


_CACHE = {}


def _get_nc():
    if "nc" not in _CACHE:
        _CACHE["nc"] = build_nc()
    return _CACHE["nc"]


def _host_prep(arrs):
    """Pack weights/biases into device layouts (bf16 x^T chunks etc.)."""
    from ml_dtypes import bfloat16 as bf

    def rpack(w):  # [512, C] -> [128, 4*C] with [p, C*r + c] = w[128r + p, c]
        c = w.shape[1]
        return np.ascontiguousarray(
            w.reshape(4, 128, c).transpose(1, 0, 2).reshape(128, 4 * c)).astype(bf)

    wq, wk, wv = arrs["Wq"], arrs["Wk"], arrs["Wv"]
    wg = np.ascontiguousarray(wq.reshape(DM, 8, 192)[:, :, 128:].reshape(DM, 512))
    bq, bk, bv = arrs["bq"], arrs["bk"], arrs["bv"]
    lam = float(arrs["lam"][0])
    li = float(arrs["lambda_init"][0])
    hl = 0.5 * (1.0 - li)

    bqk = np.zeros((128, 18), np.float32)
    for h in range(8):
        bqk[:, h] = bq[192 * h:192 * h + 128]
        bqk[:, 8 + h] = bk[128 * h:128 * h + 128]
    bqk[:, 16] = -lam
    gb = bq.reshape(8, 192)[:, 128:].reshape(512)
    rows = np.concatenate([
        np.tile(arrs["gamma"], 8) * hl,
        np.tile(arrs["beta"], 8) * hl,
        gb, bv]).astype(np.float32).reshape(1, 2048)
    g8 = np.zeros((8, 512), np.float32)
    cols = np.arange(512)
    g8[(cols % 64) // 8, cols] = 1.0

    rowsb = np.concatenate([gb, bv]).astype(bf).reshape(1, 1024)
    wqq = np.ascontiguousarray(
        wq.reshape(DM, 8, 192)[:, :, 0:128].reshape(DM, 1024))
    shared = {
        "wq": rpack(wqq), "wk": rpack(wk), "wv": rpack(wv), "wg": rpack(wg),
        "bqk": np.ascontiguousarray(bqk), "rows": rows, "rowsb": rowsb,
        "g8": g8.astype(bf),
    }
    in_maps = []
    for i in range(B):
        m = dict(shared)
        for nm, key in (("xq", "query"), ("xk", "key"), ("xv", "values")):
            m[nm] = rpack(np.ascontiguousarray(arrs[key][i].T))
        in_maps.append(m)
    return in_maps


def run(inputs, trace=False, tmpdir=None):
    from concourse.bass_utils import run_bass_kernel_spmd
    nc = _get_nc()
    arrs = {k: np.asarray(v, dtype=np.float32) for k, v in inputs.items()}
    in_maps = _host_prep(arrs)
    res = run_bass_kernel_spmd(nc, in_maps, core_ids=list(range(B)),
                               trace=trace, tmpdir=tmpdir)
    out = np.stack([res.results[i]["out"] for i in range(B)], axis=0)
    return out.astype(np.float32), res


def kernel(**inputs):
    out, _ = run(inputs)
    return out



# revision 38
# speedup vs baseline: 1.0084x; 1.0084x over previous
# Differential multi-head attention (dual softmax + GroupNorm + sigmoid gating)
# for Trainium2, batch-parallel across 8 NeuronCores (one batch row per core).
#
# Per-core math (batch b):
#   q = query @ Wq + bq -> per head: q1, q2, gate (each S x 64)
#   k = key   @ Wk + bk -> per head: k1, k2
#   v = values@ Wv + bv -> per head: v (S x 64)
#   attn = softmax(q1 k1^T / 8) - lam * softmax(q2 k2^T / 8)
#   out  = GroupNorm_{8 groups over d, reduced over (S, heads, d-in-group)}(attn @ v)
#   out  = out * (1 - lambda_init) * sigmoid(gate)
#
# Layout strategy:
#  - Host pre-packs x^T and all weights as bf16 (layout marshalling only; the
#    math runs on device).  q/k projections are d-major (q1/q2 resp. k1/k2 in
#    complementary 64-partition halves of one [128,S] tile); score matmuls are
#    K=64 at PE row-tiles 0/64 (no zero padding needed).
#  - "Flipped" o-matmul: stationary = exp-score slice [128k x 128q], moving =
#    v_aug [128k x 65] -> out is q-major [128q, 65] with the exp row-sum in
#    column 64 (ones column in v_aug).  This halves PE time vs the d-major
#    o-matmul, makes softmax normalization a per-partition scalar multiply,
#    and leaves y in the exact output layout (no final transposes).
#  - GroupNorm stats via ones-column f32r matmuls (Sum y, Sum y^2 over seq on
#    partitions), group-reduced on DVE, rsqrt via sqrt+NR, then the per-column
#    affine A,B is expanded back to [128,512] with tiny K=1/K=8 matmuls.
#  - ACT runs (nearly) only the 128 [128,1024] exp instructions + 8 tanh; all
#    projection epilogues run on DVE/Pool.  PSUM: s double-buffered (4 banks),
#    o manually packed into 3 banks (7+7+2 groups of 65 cols), proj 1 bank.

import numpy as np

B, S, H, D = 8, 1024, 8, 64
DM = H * D  # 512
NJ = S // 128   # key 128-tiles
NQ = S // 128   # query 128-blocks
EPS = 1e-3
INV = 0.125
CNT = float(S * H)  # groupnorm count per group = S * H * (D//H) / ... = 1024*8


def build_nc():
    import concourse.bacc as bacc
    import concourse.tile as tile
    from concourse import mybir

    f32 = mybir.dt.float32
    f32r = mybir.dt.float32r
    bf16 = mybir.dt.bfloat16
    AF = mybir.ActivationFunctionType
    OP = mybir.AluOpType
    AX = mybir.AxisListType

    nc = bacc.Bacc(target_bir_lowering=False)
    xq_d = nc.dram_tensor("xq", [128, 4 * S], bf16, kind="ExternalInput")
    xk_d = nc.dram_tensor("xk", [128, 4 * S], bf16, kind="ExternalInput")
    xv_d = nc.dram_tensor("xv", [128, 4 * S], bf16, kind="ExternalInput")
    wq_d = nc.dram_tensor("wq", [128, 4 * 1024], bf16, kind="ExternalInput")
    wk_d = nc.dram_tensor("wk", [128, 4 * 1024], bf16, kind="ExternalInput")
    wv_d = nc.dram_tensor("wv", [128, 4 * 512], bf16, kind="ExternalInput")
    wg_d = nc.dram_tensor("wg", [128, 4 * 512], bf16, kind="ExternalInput")
    bqk_d = nc.dram_tensor("bqk", [128, 18], f32, kind="ExternalInput")
    rows_d = nc.dram_tensor("rows", [1, 2048], f32, kind="ExternalInput")
    rowsb_d = nc.dram_tensor("rowsb", [1, 1024], bf16, kind="ExternalInput")
    g8_d = nc.dram_tensor("g8", [8, 512], bf16, kind="ExternalInput")
    out_d = nc.dram_tensor("out", [S, DM], bf16, kind="ExternalOutput")

    ts_ = nc.vector.tensor_scalar
    stt = nc.vector.scalar_tensor_tensor
    gts_ = nc.gpsimd.tensor_scalar
    gstt = nc.gpsimd.scalar_tensor_tensor

    # (t, qb) accumulation group -> (o-bank index, column offset); 65 cols per
    # group (64 d + 1 ones-sum), packed 7 + 7 + 2 into three psum banks.
    def o_loc(t, qb):
        g = t * 8 + qb
        if g < 7:
            return 0, 65 * g
        if g < 14:
            return 1, 65 * (g - 7)
        return 2, 65 * (g - 14)

    with tile.TileContext(nc) as tc:
        with tc.tile_pool(name="persist", bufs=1) as pp:
            # ---------- persistent SBUF ----------
            xq_t = pp.tile([128, 4 * S], bf16, tag="xq_t", name="xq_t")
            xk_t = pp.tile([128, 4 * S], bf16, tag="xk_t", name="xk_t")
            xv_t = pp.tile([128, 4 * S], bf16, tag="xv_t", name="xv_t")
            wq_t = pp.tile([128, 4 * 1024], bf16, tag="wq_t", name="wq_t")
            wk_t = pp.tile([128, 4 * 1024], bf16, tag="wk_t", name="wk_t")
            wv_t = pp.tile([128, 4 * 512], bf16, tag="wv_t", name="wv_t")
            wg_t = pp.tile([128, 4 * 512], bf16, tag="wg_t", name="wg_t")
            bqk_t = pp.tile([128, 18], f32, tag="bqk_t", name="bqk_t")
            rows_t = pp.tile([1, 2048], f32, tag="rows_t", name="rows_t")
            rowsb_t = pp.tile([1, 1024], bf16, tag="rowsb_t", name="rowsb_t")
            g8_t = pp.tile([8, 512], bf16, tag="g8_t", name="g8_t")
            onesb = pp.tile([128, 130], bf16, tag="onesb", name="onesb")
            onesf = pp.tile([1, 129], f32, tag="onesf", name="onesf")
            negl_t = bqk_t[:, 16:17]
            ones_c = onesb[:, 0:1]
            ones_r = onesb[0:1, 0:128]
            ones_rf = onesf[:, 0:128]
            one1 = onesf[:, 128:129]
            qp = [pp.tile([128, S], bf16, tag=f"qp{h}", name=f"qp{h}") for h in range(8)]
            kp = [pp.tile([128, S], bf16, tag=f"kp{h}", name=f"kp{h}") for h in range(8)]
            va_all = pp.tile([128, NJ, 8, 65], bf16, tag="va", name="va")
            va = [va_all[:, j] for j in range(NJ)]
            th = [pp.tile([128, 512], bf16, tag=f"th{q}", name=f"th{q}") for q in range(NQ)]
            graw = [pp.tile([128, 512], bf16, tag=f"gr{q}", name=f"gr{q}") for q in range(NQ)]
            yy = [pp.tile([128, 512], bf16, tag=f"yy{q}", name=f"yy{q}") for q in range(NQ)]
            y2b = [pp.tile([128, 512], bf16, tag=f"y2b{q}", name=f"y2b{q}") for q in range(NQ)]
            wrm = pp.tile([128, 512], bf16, tag="wrm", name="wrm")
            mrst = pp.tile([8, 2], f32, tag="mrst", name="mrst")
            mrstb = pp.tile([8, 2], bf16, tag="mrstb", name="mrstb")

            # ---------- DMAs ----------
            # Three DMA queues (SP, ACT, Pool SW-DGE) run in parallel; the
            # startup-critical set (wq/wk heads 0-1, xq both halves, xk n0)
            # is spread across them so the first scores can fire ~15us in.
            xqv = xq_t.rearrange("p (r s) -> p r s", s=S)
            xkv = xk_t.rearrange("p (r s) -> p r s", s=S)
            xvv = xv_t.rearrange("p (r s) -> p r s", s=S)
            xqd = xq_d[:, :].rearrange("p (r s) -> p r s", s=S)
            xkd = xk_d[:, :].rearrange("p (r s) -> p r s", s=S)
            xvd = xv_d[:, :].rearrange("p (r s) -> p r s", s=S)
            wqv = wq_t.rearrange("p (r c) -> p r c", c=1024)
            wkv = wk_t.rearrange("p (r c) -> p r c", c=1024)
            wqd = wq_d[:, :].rearrange("p (r c) -> p r c", c=1024)
            wkd = wk_d[:, :].rearrange("p (r c) -> p r c", c=1024)
            # Pool queue (SW DGE): merged ones/warmup/va memsets first,
            # then small consts, then mid-deadline bulk.  16 input DMAs
            # total across the queues -- the DMA-sem pool holds ~18, and
            # exceeding it serializes transfers cross-queue via sem reuse
            # (and the waiting descriptor blocks its engine queue).
            nc.gpsimd.memset(onesb, 1.0)
            nc.gpsimd.memset(onesf, 1.0)
            nc.gpsimd.memset(wrm, 0.0)
            nc.gpsimd.memset(va_all[:, :, :, 64:65], 1.0)
            nc.gpsimd.dma_start(out=bqk_t, in_=bqk_d[:, :])
            nc.gpsimd.dma_start(out=rows_t, in_=rows_d[:, :])
            nc.gpsimd.dma_start(out=rowsb_t, in_=rowsb_d[:, :])
            nc.gpsimd.dma_start(out=g8_t, in_=g8_d[:, :])
            nc.gpsimd.dma_start(out=wv_t, in_=wv_d[:, :])
            nc.gpsimd.dma_start(out=xvv[:, :, 512:1024], in_=xvd[:, :, 512:1024])
            nc.gpsimd.dma_start(out=wg_t, in_=wg_d[:, :])
            # SP queue: xq n0, xk n0/n1, xv n0 in arrival-critical order.
            nc.sync.dma_start(out=xqv[:, :, 0:512], in_=xqd[:, :, 0:512])
            nc.sync.dma_start(out=xkv[:, :, 0:512], in_=xkd[:, :, 0:512])
            nc.sync.dma_start(out=xkv[:, :, 512:1024], in_=xkd[:, :, 512:1024])
            nc.sync.dma_start(out=xvv[:, :, 0:512], in_=xvd[:, :, 0:512])
            nc.sync.dma_start(out=wqv[:, :, 256:1024], in_=wqd[:, :, 256:1024])
            # ACT queue: heads-01 weight slices + xq n1, then staged bulk.
            nc.scalar.dma_start(out=wqv[:, :, 0:256], in_=wqd[:, :, 0:256])
            nc.scalar.dma_start(out=xqv[:, :, 512:1024], in_=xqd[:, :, 512:1024])
            nc.scalar.dma_start(out=wkv[:, :, 0:256], in_=wkd[:, :, 0:256])
            nc.scalar.dma_start(out=wkv[:, :, 256:1024], in_=wkd[:, :, 256:1024])

            ghl_r = rows_t[:, 0:512]
            bhl_r = rows_t[:, 512:1024]
            gb_r = rowsb_t[:, 0:512]
            vb_r = rowsb_t[:, 512:1024]

            with tc.tile_pool(name="ps_proj", bufs=1, space="PSUM") as ps_proj, \
                 tc.tile_pool(name="ps_s", bufs=2, space="PSUM") as ps_s, \
                 tc.tile_pool(name="ps_o", bufs=1, space="PSUM") as ps_o, \
                 tc.tile_pool(name="expp", bufs=14) as expp, \
                 tc.tile_pool(name="rscp", bufs=2) as rscp:

                # ---------- projection filler units (one PE matmul each) ----
                # Projections for heads >= 2, v, and gate are emitted as
                # single-matmul units interleaved into the attention loop so
                # the PE stream stays dense while the loop is ACT(exp)-paced.
                def qk_units(h, kind, ns=(0, 1)):
                    if kind == "q":
                        w_t, x_t, dst = wq_t, xq_t, qp[h]
                        bcol = bqk_t[:, h:h + 1]
                        wofs = lambda r: 1024 * r + 128 * h
                    else:
                        w_t, x_t, dst = wk_t, xk_t, kp[h]
                        bcol = bqk_t[:, 8 + h:9 + h]
                        wofs = lambda r: 1024 * r + 128 * h
                    st = {}
                    units = []
                    for n in ns:
                        for r in range(4):
                            def f(n=n, r=r):
                                if r == 0:
                                    st[n] = ps_proj.tile(
                                        [128, 512], f32, tag="proj", name="proj")
                                nc.tensor.matmul(
                                    st[n], w_t[:, wofs(r):wofs(r) + 128],
                                    x_t[:, S * r + 512 * n:S * r + 512 * (n + 1)],
                                    start=(r == 0), stop=(r == 3))
                                if r == 3:
                                    ts_(dst[:, 512 * n:512 * (n + 1)], st[n],
                                        bcol, None, OP.add)
                            units.append(f)
                    return units

                def vg_units(q, kind):
                    # v projection (j-tile q) or gate projection (q-block q)
                    st = {}
                    units = []
                    for r in range(4):
                        def f(r=r):
                            x_t = xv_t if kind == "v" else xq_t
                            w_t = wv_t if kind == "v" else wg_t
                            if r == 0:
                                st[0] = ps_proj.tile(
                                    [128, 512], f32, tag="proj", name="proj")
                            nc.tensor.matmul(
                                st[0], x_t[:, S * r + 128 * q:S * r + 128 * (q + 1)],
                                w_t[:, 512 * r:512 * (r + 1)],
                                start=(r == 0), stop=False)
                            if r == 3:
                                if kind == "v":
                                    nc.tensor.matmul(st[0], ones_r[:], vb_r,
                                                     start=False, stop=True)
                                    nc.vector.tensor_copy(
                                        va[q][:, :, 0:64],
                                        st[0].rearrange("p (h d) -> p h d", d=64))
                                else:
                                    nc.tensor.matmul(st[0], ones_r[:], gb_r,
                                                     start=False, stop=True)
                                    nc.vector.tensor_copy(graw[q], st[0])
                        units.append(f)
                    return units

                # ---------- combine: y[:, 64h:64h+64] = o1*r1 - lam*o2*r2 --
                stats_aps = {}

                def combine(h, ot):
                    rsc = rscp.tile([128, 16], f32, tag="rsc", name="rsc")
                    nc.vector.reciprocal(
                        rsc[:, 0:7],
                        ot[0][:, 0:455].rearrange("p (g c) -> p g c", c=65)[:, :, 64])
                    nc.vector.reciprocal(
                        rsc[:, 7:14],
                        ot[1][:, 0:455].rearrange("p (g c) -> p g c", c=65)[:, :, 64])
                    nc.vector.reciprocal(
                        rsc[:, 14:16],
                        ot[2][:, 0:130].rearrange("p (g c) -> p g c", c=65)[:, :, 64])
                    ts_(rsc[:, 8:16], rsc[:, 8:16], negl_t, None, OP.mult)
                    for qb in range(8):
                        b1, c1 = o_loc(0, qb)
                        b2, c2 = o_loc(1, qb)
                        ydst = yy[qb][:, 64 * h:64 * h + 64]
                        ts_(ydst, ot[b1][:, c1:c1 + 64],
                            rsc[:, qb:qb + 1], None, OP.mult)
                        stt(ydst, ot[b2][:, c2:c2 + 64],
                            rsc[:, 8 + qb:9 + qb], ydst, OP.mult, OP.add)
                        # y^2 accumulates per head on the (mostly idle) Pool
                        # engine so the tail needs no ACT squares; head 7 on
                        # DVE so the tail stats don't wait on cross-engine
                        # sems.
                        y2eng = nc.vector if h == 7 else nc.gpsimd
                        y2eng.tensor_mul(
                            y2b[qb][:, 64 * h:64 * h + 64], ydst, ydst)
                        if h == 7:
                            # fuse GroupNorm stats into the last head's
                            # combine: sy/sy2 live in an s-pool tile (the
                            # exp stream is done with it).
                            if qb == 0:
                                stats_aps["t"] = ps_s.tile(
                                    [128, S], f32, tag="s", name="s_stats")
                            st_t = stats_aps["t"]
                            nc.tensor.matmul(st_t[0:1, 0:512], ones_c[:],
                                             yy[qb],
                                             start=(qb == 0), stop=(qb == 7))
                            nc.tensor.matmul(st_t[0:1, 512:1024], ones_c[:],
                                             y2b[qb],
                                             start=(qb == 0), stop=(qb == 7))

                # ---------- emission ----------
                # PE warmup in the DMA window: dummy matmuls into an o-pool
                # bank (idle until ~step 6) so the first real projections run
                # at full p-state.  wrm is memset first on the Pool queue, so
                # these start ~7us in, well before any input data lands.
                wps = ps_o.tile([128, 512], f32, tag="o0", name="warm")
                for i in range(8):
                    nc.tensor.matmul(wps[0:1, :], ones_c[:], wrm,
                                     start=True, stop=True)
                # Inline, ordered by DMA arrival: qp0/qp1 n0 (wq h01 + xq
                # n0), qp0 n1 (xq n1), a few more warmups to keep the PE
                # p-state hot while xk n0 lands, then kp0 n0.  Everything
                # else becomes ordered filler.
                stA = ps_s.tile([128, S], f32, tag="s", name="sA")
                for r in range(4):
                    nc.tensor.matmul(
                        stA[:, 0:512], wq_t[:, 1024 * r:1024 * r + 128],
                        xq_t[:, S * r:S * r + 512],
                        start=(r == 0), stop=(r == 3))
                ts_(qp[0][:, 0:512], stA[:, 0:512], bqk_t[:, 0:1], None, OP.add)
                for r in range(4):
                    nc.tensor.matmul(
                        stA[:, 512:1024], wq_t[:, 1024 * r + 128:1024 * r + 256],
                        xq_t[:, S * r:S * r + 512],
                        start=(r == 0), stop=(r == 3))
                ts_(qp[1][:, 0:512], stA[:, 512:1024], bqk_t[:, 1:2], None,
                    OP.add)
                stB = ps_s.tile([128, S], f32, tag="s", name="sB")
                for r in range(4):
                    nc.tensor.matmul(
                        stB[:, 0:512], wq_t[:, 1024 * r:1024 * r + 128],
                        xq_t[:, S * r + 512:S * r + 1024],
                        start=(r == 0), stop=(r == 3))
                ts_(qp[0][:, 512:1024], stB[:, 0:512], bqk_t[:, 0:1], None,
                    OP.add)
                for i in range(4):
                    nc.tensor.matmul(wps[0:1, :], ones_c[:], wrm,
                                     start=True, stop=True)
                for r in range(4):
                    nc.tensor.matmul(
                        stB[:, 512:1024], wk_t[:, 1024 * r:1024 * r + 128],
                        xk_t[:, S * r:S * r + 512],
                        start=(r == 0), stop=(r == 3))
                ts_(kp[0][:, 0:512], stB[:, 512:1024], bqk_t[:, 8:9], None,
                    OP.add)

                filler = []
                flabels = []

                def fext(units, label):
                    filler.extend(units)
                    flabels.extend([label] * len(units))

                fext(qk_units(1, "k", ns=(0,)), "kp1n0")
                fext(qk_units(0, "k", ns=(1,)), "kp0n1")
                fext(qk_units(1, "q", ns=(1,)), "qp1n1")
                fext(qk_units(1, "k", ns=(1,)), "kp1n1")
                for j in range(NJ):
                    fext(vg_units(j, "v"), f"v{j}")
                for h2 in range(2, 8):
                    fext(qk_units(h2, "q"), f"qp{h2}")
                    fext(qk_units(h2, "k"), f"kp{h2}")
                for q in range(NQ):
                    fext(vg_units(q, "g"), f"g{q}")
                fdone = []

                # software-pipelined attention loop with a 2-step o-lag:
                # everything PE does in step i depends only on exp(i-2), so
                # the PE never waits mid-step and the exp stream stays dense.
                steps = [(h, t, j) for h in range(8) for t in range(2)
                         for j in range(NJ)]
                otiles = {}
                pending = []

                def emit_o(ph, pt, pj, pex):
                    # program-order safety: the v epilogue writing va[pj]
                    # must already be issued, else no hazard dep exists and
                    # the PE reads garbage (race seen on hw).
                    assert fdone.count(f"v{pj}") == 4 or ph > 0 or pt > 0, \
                        f"va[{pj}] written {fdone.count(f'v{pj}')}/4 at o-emit"
                    if pt == 0 and pj == 0:
                        otiles[ph] = [
                            ps_o.tile([128, 512], f32, tag=f"o{i}", name=f"o{i}")
                            for i in range(3)]
                    ot = otiles[ph]
                    for qb in range(NQ):
                        bi, c0 = o_loc(pt, qb)
                        # start only on the FIRST matmul into each bank this
                        # head: start_tensor_calc zeroes the whole bank, so
                        # later groups accumulate onto the zeroed regions.
                        first = (pj == 0) and (pt, qb) in ((0, 0), (0, 7), (1, 6))
                        nc.tensor.matmul(
                            ot[bi][:, c0:c0 + 65],
                            pex[:, 128 * qb:128 * (qb + 1)],
                            va[pj][:, ph, :],
                            start=first, stop=(pj == NJ - 1),
                            skip_group_check=True)
                    if pt == 1 and pj == NJ - 1:
                        combine(ph, ot)

                def filler_target(i):
                    # cumulative filler units to have popped by end of step i.
                    # Front-loaded (3/step) while the PE has no o-work, so
                    # every v-unit's va write is ISSUED (program order) well
                    # before the first o-matmul that reads it (o-emits start
                    # at idx 13); 1/step after, so each proj group spans 4
                    # steps and its DVE epilogue never blocks the next
                    # group's psum-bank reuse.
                    c = 3 * min(i + 1, 20)
                    c += 2 * max(0, min(i + 1, 32) - 20)
                    c += max(0, i + 1 - 32)
                    return c

                fpopped = 0
                for idx, (h, t, j) in enumerate(steps):
                    # o-matmuls of step i-2 first: they are ready the moment
                    # exp(i-1) starts, so their leading stationary-switch
                    # stall hides under exp; scores of step i follow and
                    # complete well before exp(i-1) ends.
                    odepth = 13 if idx < 24 else max(2, 13 - (idx - 23) // 3)
                    while len(pending) >= odepth:
                        emit_o(*pending.pop(0))
                    s_t = ps_s.tile([128, S], f32, tag="s", name="s")
                    for n in range(2):
                        nc.tensor.matmul(
                            s_t[:, 512 * n:512 * (n + 1)],
                            kp[h][64 * t:64 * t + 64, 128 * j:128 * (j + 1)],
                            qp[h][64 * t:64 * t + 64, 512 * n:512 * (n + 1)],
                            start=True, stop=True)
                    ex = expp.tile([128, S], bf16, tag="ex", name="ex")
                    nc.scalar.activation(ex, s_t, AF.Exp, scale=INV)
                    pending.append((h, t, j, ex))
                    while filler and fpopped < filler_target(idx):
                        filler.pop(0)()
                        fdone.append(flabels[fpopped])
                        fpopped += 1
                while pending:
                    emit_o(*pending.pop(0))
                while filler:
                    filler.pop(0)()

            # ---------- tail: GroupNorm stats + affine + gate + output ----
            with tc.tile_pool(name="ps_tail", bufs=1, space="PSUM") as ps_t, \
                 tc.tile_pool(name="oqp", bufs=8) as oqp, \
                 tc.tile_pool(name="tsb", bufs=1) as tsb:

                for q in range(NQ):
                    nc.scalar.activation(th[q], graw[q], AF.Tanh, scale=0.5)
                mcol = ps_t.tile([8, 2], f32, tag="mcol", name="mcol")
                rxA = ps_t.tile([1, 512], f32, tag="rxA", name="rxA")
                rxB = ps_t.tile([1, 512], f32, tag="rxB", name="rxB")
                ab = ps_t.tile([128, 1024], f32, tag="ab", name="ab")

                sy = stats_aps["t"][0:1, 0:512]
                sy2 = stats_aps["t"][0:1, 512:1024]
                gsum = tsb.tile([1, 8], f32, tag="gsum", name="gsum")
                g2 = tsb.tile([1, 8], f32, tag="g2", name="g2")
                nc.vector.tensor_reduce(
                    gsum, sy.rearrange("o (h g e) -> o g h e", h=8, g=8),
                    axis=AX.XY, op=OP.add)
                nc.vector.tensor_reduce(
                    g2, sy2.rearrange("o (h g e) -> o g h e", h=8, g=8),
                    axis=AX.XY, op=OP.add)
                nc.tensor.matmul(mcol[:, 0:1], gsum, one1, start=True, stop=False,
                                 skip_group_check=True)
                nc.tensor.matmul(mcol[:, 1:2], g2, one1, start=False, stop=True,
                                 skip_group_check=True)

                e2t = tsb.tile([8, 1], f32, tag="e2t", name="e2t")
                nm = tsb.tile([8, 1], f32, tag="nm", name="nm")
                veps = tsb.tile([8, 1], f32, tag="veps", name="veps")
                rr = tsb.tile([8, 1], f32, tag="rr", name="rr")
                rsd = tsb.tile([8, 1], f32, tag="rsd", name="rsd")
                ts_(mrst[:, 1:2], mcol[:, 0:1], 1.0 / float(S * H * 8), None, OP.mult)
                ts_(e2t, mcol[:, 1:2], 1.0 / float(S * H * 8), None, OP.mult)
                ts_(nm, mrst[:, 1:2], mrst[:, 1:2], -1.0, OP.mult, OP.mult)
                stt(veps, nm, EPS, e2t, OP.add, OP.add)
                # rsqrt via bit-trick seed + 2 Newton iterations (DVE only,
                # avoids the ACT sqrt table load)
                vi = veps.bitcast(mybir.dt.int32)
                si = rsd.bitcast(mybir.dt.int32)
                ts_(si, vi, 1, None, OP.logical_shift_right)
                ts_(si, si, -1, None, OP.bitwise_xor)
                ts_(si, si, 0x5F3759E0, None, OP.add)
                for _ in range(2):
                    nc.vector.tensor_mul(rr, rsd, rsd)
                    nc.vector.tensor_mul(rr, rr, veps)
                    ts_(rr, rr, -0.5, 1.5, OP.mult, OP.add)
                    nc.vector.tensor_mul(rsd, rsd, rr)
                nc.vector.tensor_copy(mrst[:, 0:1], rsd)

                nc.vector.tensor_copy(mrstb, mrst)
                nc.tensor.matmul(rxA, mrstb[:, 0:1], g8_t[:, :], start=True, stop=True)
                nc.tensor.matmul(rxB, mrstb[:, 1:2], g8_t[:, :], start=True, stop=True)
                arow = tsb.tile([1, 512], bf16, tag="arow", name="arow")
                btmp = tsb.tile([1, 512], f32, tag="btmp", name="btmp")
                brow = tsb.tile([1, 512], bf16, tag="brow", name="brow")
                nc.vector.tensor_mul(arow, ghl_r, rxA)
                nc.vector.tensor_mul(btmp, rxB, arow)
                nc.vector.tensor_sub(brow, bhl_r, btmp)
                nc.tensor.matmul(ab[:, 0:512], ones_r[:], arow,
                                 start=True, stop=True)
                nc.tensor.matmul(ab[:, 512:1024], ones_r[:], brow,
                                 start=True, stop=True)

                ab_sb = tsb.tile([128, 1024], bf16, tag="ab_sb", name="ab_sb")
                nc.vector.tensor_copy(ab_sb[:, 0:512], ab[:, 0:512])
                nc.vector.tensor_copy(ab_sb[:, 512:1024], ab[:, 512:1024])
                # finalize on DVE only (cross-engine DVE/Pool splitting was
                # measured ~2x slower per op: SBUF port contention + sem
                # ping-pong); output DMA round-robins all three queues.
                dmas = (nc.sync.dma_start, nc.scalar.dma_start,
                        nc.gpsimd.dma_start)
                for qb in range(NQ):
                    oq = oqp.tile([128, 512], bf16, tag="oq", name="oq")
                    nc.vector.tensor_mul(oq, yy[qb], ab_sb[:, 0:512])
                    nc.vector.tensor_add(oq, oq, ab_sb[:, 512:1024])
                    stt(oq, th[qb], 1.0, oq, OP.add, OP.mult)
                    dmas[qb % 3](out=out_d[128 * qb:128 * (qb + 1), :], in_=oq)

    nc.finalize()
    return nc


_CACHE = {}


def _get_nc():
    if "nc" not in _CACHE:
        _CACHE["nc"] = build_nc()
    return _CACHE["nc"]


def _host_prep(arrs):
    """Pack weights/biases into device layouts (bf16 x^T chunks etc.)."""
    from ml_dtypes import bfloat16 as bf

    def rpack(w):  # [512, C] -> [128, 4*C] with [p, C*r + c] = w[128r + p, c]
        c = w.shape[1]
        return np.ascontiguousarray(
            w.reshape(4, 128, c).transpose(1, 0, 2).reshape(128, 4 * c)).astype(bf)

    wq, wk, wv = arrs["Wq"], arrs["Wk"], arrs["Wv"]
    wg = np.ascontiguousarray(wq.reshape(DM, 8, 192)[:, :, 128:].reshape(DM, 512))
    bq, bk, bv = arrs["bq"], arrs["bk"], arrs["bv"]
    lam = float(arrs["lam"][0])
    li = float(arrs["lambda_init"][0])
    hl = 0.5 * (1.0 - li)

    bqk = np.zeros((128, 18), np.float32)
    for h in range(8):
        bqk[:, h] = bq[192 * h:192 * h + 128]
        bqk[:, 8 + h] = bk[128 * h:128 * h + 128]
    bqk[:, 16] = -lam
    gb = bq.reshape(8, 192)[:, 128:].reshape(512)
    rows = np.concatenate([
        np.tile(arrs["gamma"], 8) * hl,
        np.tile(arrs["beta"], 8) * hl,
        gb, bv]).astype(np.float32).reshape(1, 2048)
    g8 = np.zeros((8, 512), np.float32)
    cols = np.arange(512)
    g8[(cols % 64) // 8, cols] = 1.0

    rowsb = np.concatenate([gb, bv]).astype(bf).reshape(1, 1024)
    wqq = np.ascontiguousarray(
        wq.reshape(DM, 8, 192)[:, :, 0:128].reshape(DM, 1024))
    shared = {
        "wq": rpack(wqq), "wk": rpack(wk), "wv": rpack(wv), "wg": rpack(wg),
        "bqk": np.ascontiguousarray(bqk), "rows": rows, "rowsb": rowsb,
        "g8": g8.astype(bf),
    }
    in_maps = []
    for i in range(B):
        m = dict(shared)
        for nm, key in (("xq", "query"), ("xk", "key"), ("xv", "values")):
            m[nm] = rpack(np.ascontiguousarray(arrs[key][i].T))
        in_maps.append(m)
    return in_maps


def run(inputs, trace=False, tmpdir=None):
    from concourse.bass_utils import run_bass_kernel_spmd
    nc = _get_nc()
    arrs = {k: np.asarray(v, dtype=np.float32) for k, v in inputs.items()}
    in_maps = _host_prep(arrs)
    res = run_bass_kernel_spmd(nc, in_maps, core_ids=list(range(B)),
                               trace=trace, tmpdir=tmpdir)
    out = np.stack([res.results[i]["out"] for i in range(B)], axis=0)
    return out.astype(np.float32), res


def kernel(**inputs):
    out, _ = run(inputs)
    return out



# revision 39
# speedup vs baseline: 1.0122x; 1.0037x over previous
# Differential multi-head attention (dual softmax + GroupNorm + sigmoid gating)
# for Trainium2, batch-parallel across 8 NeuronCores (one batch row per core).
#
# Per-core math (batch b):
#   q = query @ Wq + bq -> per head: q1, q2, gate (each S x 64)
#   k = key   @ Wk + bk -> per head: k1, k2
#   v = values@ Wv + bv -> per head: v (S x 64)
#   attn = softmax(q1 k1^T / 8) - lam * softmax(q2 k2^T / 8)
#   out  = GroupNorm_{8 groups over d, reduced over (S, heads, d-in-group)}(attn @ v)
#   out  = out * (1 - lambda_init) * sigmoid(gate)
#
# Layout strategy:
#  - Host pre-packs x^T and all weights as bf16 (layout marshalling only; the
#    math runs on device).  q/k projections are d-major (q1/q2 resp. k1/k2 in
#    complementary 64-partition halves of one [128,S] tile); score matmuls are
#    K=64 at PE row-tiles 0/64 (no zero padding needed).
#  - "Flipped" o-matmul: stationary = exp-score slice [128k x 128q], moving =
#    v_aug [128k x 65] -> out is q-major [128q, 65] with the exp row-sum in
#    column 64 (ones column in v_aug).  This halves PE time vs the d-major
#    o-matmul, makes softmax normalization a per-partition scalar multiply,
#    and leaves y in the exact output layout (no final transposes).
#  - GroupNorm stats via ones-column f32r matmuls (Sum y, Sum y^2 over seq on
#    partitions), group-reduced on DVE, rsqrt via sqrt+NR, then the per-column
#    affine A,B is expanded back to [128,512] with tiny K=1/K=8 matmuls.
#  - ACT runs (nearly) only the 128 [128,1024] exp instructions + 8 tanh; all
#    projection epilogues run on DVE/Pool.  PSUM: s double-buffered (4 banks),
#    o manually packed into 3 banks (7+7+2 groups of 65 cols), proj 1 bank.

import numpy as np

B, S, H, D = 8, 1024, 8, 64
DM = H * D  # 512
NJ = S // 128   # key 128-tiles
NQ = S // 128   # query 128-blocks
EPS = 1e-3
INV = 0.125
CNT = float(S * H)  # groupnorm count per group = S * H * (D//H) / ... = 1024*8


def build_nc():
    import concourse.bacc as bacc
    import concourse.tile as tile
    from concourse import mybir

    f32 = mybir.dt.float32
    f32r = mybir.dt.float32r
    bf16 = mybir.dt.bfloat16
    AF = mybir.ActivationFunctionType
    OP = mybir.AluOpType
    AX = mybir.AxisListType

    nc = bacc.Bacc(target_bir_lowering=False)
    xq_d = nc.dram_tensor("xq", [128, 4 * S], bf16, kind="ExternalInput")
    xk_d = nc.dram_tensor("xk", [128, 4 * S], bf16, kind="ExternalInput")
    xv_d = nc.dram_tensor("xv", [128, 4 * S], bf16, kind="ExternalInput")
    wq_d = nc.dram_tensor("wq", [128, 4 * 1024], bf16, kind="ExternalInput")
    wk_d = nc.dram_tensor("wk", [128, 4 * 1024], bf16, kind="ExternalInput")
    wv_d = nc.dram_tensor("wv", [128, 4 * 512], bf16, kind="ExternalInput")
    wg_d = nc.dram_tensor("wg", [128, 4 * 512], bf16, kind="ExternalInput")
    bqk_d = nc.dram_tensor("bqk", [128, 18], f32, kind="ExternalInput")
    rows_d = nc.dram_tensor("rows", [1, 2048], f32, kind="ExternalInput")
    rowsb_d = nc.dram_tensor("rowsb", [1, 1024], bf16, kind="ExternalInput")
    g8_d = nc.dram_tensor("g8", [8, 512], bf16, kind="ExternalInput")
    out_d = nc.dram_tensor("out", [S, DM], bf16, kind="ExternalOutput")

    ts_ = nc.vector.tensor_scalar
    stt = nc.vector.scalar_tensor_tensor
    gts_ = nc.gpsimd.tensor_scalar
    gstt = nc.gpsimd.scalar_tensor_tensor

    # (t, qb) accumulation group -> (o-bank index, column offset); 65 cols per
    # group (64 d + 1 ones-sum), packed 7 + 7 + 2 into three psum banks.
    def o_loc(t, qb):
        g = t * 8 + qb
        if g < 7:
            return 0, 65 * g
        if g < 14:
            return 1, 65 * (g - 7)
        return 2, 65 * (g - 14)

    with tile.TileContext(nc) as tc:
        with tc.tile_pool(name="persist", bufs=1) as pp:
            # ---------- persistent SBUF ----------
            xq_t = pp.tile([128, 4 * S], bf16, tag="xq_t", name="xq_t")
            xk_t = pp.tile([128, 4 * S], bf16, tag="xk_t", name="xk_t")
            xv_t = pp.tile([128, 4 * S], bf16, tag="xv_t", name="xv_t")
            wq_t = pp.tile([128, 4 * 1024], bf16, tag="wq_t", name="wq_t")
            wk_t = pp.tile([128, 4 * 1024], bf16, tag="wk_t", name="wk_t")
            wv_t = pp.tile([128, 4 * 512], bf16, tag="wv_t", name="wv_t")
            wg_t = pp.tile([128, 4 * 512], bf16, tag="wg_t", name="wg_t")
            bqk_t = pp.tile([128, 18], f32, tag="bqk_t", name="bqk_t")
            rows_t = pp.tile([1, 2048], f32, tag="rows_t", name="rows_t")
            rowsb_t = pp.tile([1, 1024], bf16, tag="rowsb_t", name="rowsb_t")
            g8_t = pp.tile([8, 512], bf16, tag="g8_t", name="g8_t")
            onesb = pp.tile([128, 130], bf16, tag="onesb", name="onesb")
            onesf = pp.tile([1, 129], f32, tag="onesf", name="onesf")
            negl_t = bqk_t[:, 16:17]
            ones_c = onesb[:, 0:1]
            ones_r = onesb[0:1, 0:128]
            ones_rf = onesf[:, 0:128]
            one1 = onesf[:, 128:129]
            qp = [pp.tile([128, S], bf16, tag=f"qp{h}", name=f"qp{h}") for h in range(8)]
            kp = [pp.tile([128, S], bf16, tag=f"kp{h}", name=f"kp{h}") for h in range(8)]
            va_all = pp.tile([128, NJ, 8, 65], bf16, tag="va", name="va")
            va = [va_all[:, j] for j in range(NJ)]
            th = [pp.tile([128, 512], bf16, tag=f"th{q}", name=f"th{q}") for q in range(NQ)]
            graw = [pp.tile([128, 512], bf16, tag=f"gr{q}", name=f"gr{q}") for q in range(NQ)]
            yy = [pp.tile([128, 512], bf16, tag=f"yy{q}", name=f"yy{q}") for q in range(NQ)]
            y2b = [pp.tile([128, 512], bf16, tag=f"y2b{q}", name=f"y2b{q}") for q in range(NQ)]
            wrm = pp.tile([128, 512], bf16, tag="wrm", name="wrm")
            mrst = pp.tile([8, 2], f32, tag="mrst", name="mrst")
            mrstb = pp.tile([8, 2], bf16, tag="mrstb", name="mrstb")

            # ---------- DMAs ----------
            # Three DMA queues (SP, ACT, Pool SW-DGE) run in parallel; the
            # startup-critical set (wq/wk heads 0-1, xq both halves, xk n0)
            # is spread across them so the first scores can fire ~15us in.
            xqv = xq_t.rearrange("p (r s) -> p r s", s=S)
            xkv = xk_t.rearrange("p (r s) -> p r s", s=S)
            xvv = xv_t.rearrange("p (r s) -> p r s", s=S)
            xqd = xq_d[:, :].rearrange("p (r s) -> p r s", s=S)
            xkd = xk_d[:, :].rearrange("p (r s) -> p r s", s=S)
            xvd = xv_d[:, :].rearrange("p (r s) -> p r s", s=S)
            wqv = wq_t.rearrange("p (r c) -> p r c", c=1024)
            wkv = wk_t.rearrange("p (r c) -> p r c", c=1024)
            wqd = wq_d[:, :].rearrange("p (r c) -> p r c", c=1024)
            wkd = wk_d[:, :].rearrange("p (r c) -> p r c", c=1024)
            # Pool queue (SW DGE): merged ones/warmup/va memsets first,
            # then small consts, then mid-deadline bulk.  16 input DMAs
            # total across the queues -- the DMA-sem pool holds ~18, and
            # exceeding it serializes transfers cross-queue via sem reuse
            # (and the waiting descriptor blocks its engine queue).
            nc.gpsimd.memset(onesb, 1.0)
            nc.gpsimd.memset(onesf, 1.0)
            nc.gpsimd.memset(wrm, 0.0)
            nc.gpsimd.memset(va_all[:, :, :, 64:65], 1.0)
            nc.gpsimd.dma_start(out=bqk_t, in_=bqk_d[:, :])
            nc.gpsimd.dma_start(out=wkv[:, :, 0:256], in_=wkd[:, :, 0:256])
            nc.gpsimd.dma_start(out=rows_t, in_=rows_d[:, :])
            nc.gpsimd.dma_start(out=rowsb_t, in_=rowsb_d[:, :])
            nc.gpsimd.dma_start(out=g8_t, in_=g8_d[:, :])
            nc.gpsimd.dma_start(out=xvv[:, :, 512:1024], in_=xvd[:, :, 512:1024])
            nc.gpsimd.dma_start(out=wg_t, in_=wg_d[:, :])
            # SP queue: xq n0, xk n0/n1, xv n0 in arrival-critical order.
            nc.sync.dma_start(out=xqv[:, :, 0:512], in_=xqd[:, :, 0:512])
            nc.sync.dma_start(out=xkv[:, :, 0:512], in_=xkd[:, :, 0:512])
            nc.sync.dma_start(out=xkv[:, :, 512:1024], in_=xkd[:, :, 512:1024])
            nc.sync.dma_start(out=wqv[:, :, 256:1024], in_=wqd[:, :, 256:1024])
            # ACT queue: heads-01 weight slices + xq n1, then staged bulk.
            nc.scalar.dma_start(out=wqv[:, :, 0:256], in_=wqd[:, :, 0:256])
            nc.scalar.dma_start(out=xqv[:, :, 512:1024], in_=xqd[:, :, 512:1024])
            nc.scalar.dma_start(out=wv_t, in_=wv_d[:, :])
            nc.scalar.dma_start(out=xvv[:, :, 0:512], in_=xvd[:, :, 0:512])
            nc.scalar.dma_start(out=wkv[:, :, 256:1024], in_=wkd[:, :, 256:1024])

            ghl_r = rows_t[:, 0:512]
            bhl_r = rows_t[:, 512:1024]
            gb_r = rowsb_t[:, 0:512]
            vb_r = rowsb_t[:, 512:1024]

            with tc.tile_pool(name="ps_proj", bufs=1, space="PSUM") as ps_proj, \
                 tc.tile_pool(name="ps_s", bufs=2, space="PSUM") as ps_s, \
                 tc.tile_pool(name="ps_o", bufs=1, space="PSUM") as ps_o, \
                 tc.tile_pool(name="expp", bufs=14) as expp, \
                 tc.tile_pool(name="rscp", bufs=2) as rscp:

                # ---------- projection filler units (one PE matmul each) ----
                # Projections for heads >= 2, v, and gate are emitted as
                # single-matmul units interleaved into the attention loop so
                # the PE stream stays dense while the loop is ACT(exp)-paced.
                def qk_units(h, kind, ns=(0, 1)):
                    if kind == "q":
                        w_t, x_t, dst = wq_t, xq_t, qp[h]
                        bcol = bqk_t[:, h:h + 1]
                        wofs = lambda r: 1024 * r + 128 * h
                    else:
                        w_t, x_t, dst = wk_t, xk_t, kp[h]
                        bcol = bqk_t[:, 8 + h:9 + h]
                        wofs = lambda r: 1024 * r + 128 * h
                    st = {}
                    units = []
                    for n in ns:
                        for r in range(4):
                            def f(n=n, r=r):
                                if r == 0:
                                    st[n] = ps_proj.tile(
                                        [128, 512], f32, tag="proj", name="proj")
                                nc.tensor.matmul(
                                    st[n], w_t[:, wofs(r):wofs(r) + 128],
                                    x_t[:, S * r + 512 * n:S * r + 512 * (n + 1)],
                                    start=(r == 0), stop=(r == 3))
                                if r == 3:
                                    ts_(dst[:, 512 * n:512 * (n + 1)], st[n],
                                        bcol, None, OP.add)
                            units.append(f)
                    return units

                def vg_units(q, kind):
                    # v projection (j-tile q) or gate projection (q-block q)
                    st = {}
                    units = []
                    for r in range(4):
                        def f(r=r):
                            x_t = xv_t if kind == "v" else xq_t
                            w_t = wv_t if kind == "v" else wg_t
                            if r == 0:
                                st[0] = ps_proj.tile(
                                    [128, 512], f32, tag="proj", name="proj")
                            nc.tensor.matmul(
                                st[0], x_t[:, S * r + 128 * q:S * r + 128 * (q + 1)],
                                w_t[:, 512 * r:512 * (r + 1)],
                                start=(r == 0), stop=False)
                            if r == 3:
                                if kind == "v":
                                    nc.tensor.matmul(st[0], ones_r[:], vb_r,
                                                     start=False, stop=True)
                                    nc.vector.tensor_copy(
                                        va[q][:, :, 0:64],
                                        st[0].rearrange("p (h d) -> p h d", d=64))
                                else:
                                    nc.tensor.matmul(st[0], ones_r[:], gb_r,
                                                     start=False, stop=True)
                                    nc.vector.tensor_copy(graw[q], st[0])
                        units.append(f)
                    return units

                # ---------- combine: y[:, 64h:64h+64] = o1*r1 - lam*o2*r2 --
                stats_aps = {}

                def combine(h, ot):
                    rsc = rscp.tile([128, 16], f32, tag="rsc", name="rsc")
                    nc.vector.reciprocal(
                        rsc[:, 0:7],
                        ot[0][:, 0:455].rearrange("p (g c) -> p g c", c=65)[:, :, 64])
                    nc.vector.reciprocal(
                        rsc[:, 7:14],
                        ot[1][:, 0:455].rearrange("p (g c) -> p g c", c=65)[:, :, 64])
                    nc.vector.reciprocal(
                        rsc[:, 14:16],
                        ot[2][:, 0:130].rearrange("p (g c) -> p g c", c=65)[:, :, 64])
                    ts_(rsc[:, 8:16], rsc[:, 8:16], negl_t, None, OP.mult)
                    for qb in range(8):
                        b1, c1 = o_loc(0, qb)
                        b2, c2 = o_loc(1, qb)
                        ydst = yy[qb][:, 64 * h:64 * h + 64]
                        ts_(ydst, ot[b1][:, c1:c1 + 64],
                            rsc[:, qb:qb + 1], None, OP.mult)
                        stt(ydst, ot[b2][:, c2:c2 + 64],
                            rsc[:, 8 + qb:9 + qb], ydst, OP.mult, OP.add)
                        # y^2 accumulates per head on the (mostly idle) Pool
                        # engine so the tail needs no ACT squares; head 7 on
                        # DVE so the tail stats don't wait on cross-engine
                        # sems.
                        y2eng = nc.vector if h == 7 else nc.gpsimd
                        y2eng.tensor_mul(
                            y2b[qb][:, 64 * h:64 * h + 64], ydst, ydst)
                        if h == 7:
                            # fuse GroupNorm stats into the last head's
                            # combine: sy/sy2 live in an s-pool tile (the
                            # exp stream is done with it).
                            if qb == 0:
                                stats_aps["t"] = ps_s.tile(
                                    [128, S], f32, tag="s", name="s_stats")
                            st_t = stats_aps["t"]
                            nc.tensor.matmul(st_t[0:1, 0:512], ones_c[:],
                                             yy[qb],
                                             start=(qb == 0), stop=(qb == 7))
                            nc.tensor.matmul(st_t[0:1, 512:1024], ones_c[:],
                                             y2b[qb],
                                             start=(qb == 0), stop=(qb == 7))

                # ---------- emission ----------
                # PE warmup in the DMA window: dummy matmuls into an o-pool
                # bank (idle until ~step 6) so the first real projections run
                # at full p-state.  wrm is memset first on the Pool queue, so
                # these start ~7us in, well before any input data lands.
                wps = ps_o.tile([128, 512], f32, tag="o0", name="warm")
                for i in range(8):
                    nc.tensor.matmul(wps[0:1, :], ones_c[:], wrm,
                                     start=True, stop=True)
                # Inline, ordered by DMA arrival: qp0/qp1 n0 (wq h01 + xq
                # n0), qp0 n1 (xq n1), a few more warmups to keep the PE
                # p-state hot while xk n0 lands, then kp0 n0.  Everything
                # else becomes ordered filler.
                stA = ps_s.tile([128, S], f32, tag="s", name="sA")
                for r in range(4):
                    nc.tensor.matmul(
                        stA[:, 0:512], wq_t[:, 1024 * r:1024 * r + 128],
                        xq_t[:, S * r:S * r + 512],
                        start=(r == 0), stop=(r == 3))
                ts_(qp[0][:, 0:512], stA[:, 0:512], bqk_t[:, 0:1], None, OP.add)
                for r in range(4):
                    nc.tensor.matmul(
                        stA[:, 512:1024], wq_t[:, 1024 * r + 128:1024 * r + 256],
                        xq_t[:, S * r:S * r + 512],
                        start=(r == 0), stop=(r == 3))
                ts_(qp[1][:, 0:512], stA[:, 512:1024], bqk_t[:, 1:2], None,
                    OP.add)
                stB = ps_s.tile([128, S], f32, tag="s", name="sB")
                for r in range(4):
                    nc.tensor.matmul(
                        stB[:, 0:512], wq_t[:, 1024 * r:1024 * r + 128],
                        xq_t[:, S * r + 512:S * r + 1024],
                        start=(r == 0), stop=(r == 3))
                ts_(qp[0][:, 512:1024], stB[:, 0:512], bqk_t[:, 0:1], None,
                    OP.add)
                for i in range(4):
                    nc.tensor.matmul(wps[0:1, :], ones_c[:], wrm,
                                     start=True, stop=True)
                for r in range(4):
                    nc.tensor.matmul(
                        stB[:, 512:1024], wk_t[:, 1024 * r:1024 * r + 128],
                        xk_t[:, S * r:S * r + 512],
                        start=(r == 0), stop=(r == 3))
                ts_(kp[0][:, 0:512], stB[:, 512:1024], bqk_t[:, 8:9], None,
                    OP.add)

                filler = []
                flabels = []

                def fext(units, label):
                    filler.extend(units)
                    flabels.extend([label] * len(units))

                fext(qk_units(1, "k", ns=(0,)), "kp1n0")
                fext(qk_units(0, "k", ns=(1,)), "kp0n1")
                fext(qk_units(1, "q", ns=(1,)), "qp1n1")
                fext(qk_units(1, "k", ns=(1,)), "kp1n1")
                for j in range(NJ):
                    fext(vg_units(j, "v"), f"v{j}")
                for h2 in range(2, 8):
                    fext(qk_units(h2, "q"), f"qp{h2}")
                    fext(qk_units(h2, "k"), f"kp{h2}")
                for q in range(NQ):
                    fext(vg_units(q, "g"), f"g{q}")
                fdone = []

                # software-pipelined attention loop with a 2-step o-lag:
                # everything PE does in step i depends only on exp(i-2), so
                # the PE never waits mid-step and the exp stream stays dense.
                steps = [(h, t, j) for h in range(8) for t in range(2)
                         for j in range(NJ)]
                otiles = {}
                pending = []

                def emit_o(ph, pt, pj, pex):
                    # program-order safety: the v epilogue writing va[pj]
                    # must already be issued, else no hazard dep exists and
                    # the PE reads garbage (race seen on hw).
                    assert fdone.count(f"v{pj}") == 4 or ph > 0 or pt > 0, \
                        f"va[{pj}] written {fdone.count(f'v{pj}')}/4 at o-emit"
                    if pt == 0 and pj == 0:
                        otiles[ph] = [
                            ps_o.tile([128, 512], f32, tag=f"o{i}", name=f"o{i}")
                            for i in range(3)]
                    ot = otiles[ph]
                    for qb in range(NQ):
                        bi, c0 = o_loc(pt, qb)
                        # start only on the FIRST matmul into each bank this
                        # head: start_tensor_calc zeroes the whole bank, so
                        # later groups accumulate onto the zeroed regions.
                        first = (pj == 0) and (pt, qb) in ((0, 0), (0, 7), (1, 6))
                        nc.tensor.matmul(
                            ot[bi][:, c0:c0 + 65],
                            pex[:, 128 * qb:128 * (qb + 1)],
                            va[pj][:, ph, :],
                            start=first, stop=(pj == NJ - 1),
                            skip_group_check=True)
                    if pt == 1 and pj == NJ - 1:
                        combine(ph, ot)

                def filler_target(i):
                    # cumulative filler units to have popped by end of step i.
                    # Front-loaded (3/step) while the PE has no o-work, so
                    # every v-unit's va write is ISSUED (program order) well
                    # before the first o-matmul that reads it (o-emits start
                    # at idx 13); 1/step after, so each proj group spans 4
                    # steps and its DVE epilogue never blocks the next
                    # group's psum-bank reuse.
                    c = 3 * min(i + 1, 20)
                    c += 2 * max(0, min(i + 1, 32) - 20)
                    c += max(0, i + 1 - 32)
                    return c

                fpopped = 0
                for idx, (h, t, j) in enumerate(steps):
                    # o-matmuls of step i-2 first: they are ready the moment
                    # exp(i-1) starts, so their leading stationary-switch
                    # stall hides under exp; scores of step i follow and
                    # complete well before exp(i-1) ends.
                    odepth = 13 if idx < 24 else max(2, 13 - (idx - 23) // 3)
                    while len(pending) >= odepth:
                        emit_o(*pending.pop(0))
                    s_t = ps_s.tile([128, S], f32, tag="s", name="s")
                    for n in range(2):
                        nc.tensor.matmul(
                            s_t[:, 512 * n:512 * (n + 1)],
                            kp[h][64 * t:64 * t + 64, 128 * j:128 * (j + 1)],
                            qp[h][64 * t:64 * t + 64, 512 * n:512 * (n + 1)],
                            start=True, stop=True)
                    ex = expp.tile([128, S], bf16, tag="ex", name="ex")
                    nc.scalar.activation(ex, s_t, AF.Exp, scale=INV)
                    pending.append((h, t, j, ex))
                    while filler and fpopped < filler_target(idx):
                        filler.pop(0)()
                        fdone.append(flabels[fpopped])
                        fpopped += 1
                while pending:
                    emit_o(*pending.pop(0))
                while filler:
                    filler.pop(0)()

            # ---------- tail: GroupNorm stats + affine + gate + output ----
            with tc.tile_pool(name="ps_tail", bufs=1, space="PSUM") as ps_t, \
                 tc.tile_pool(name="oqp", bufs=8) as oqp, \
                 tc.tile_pool(name="tsb", bufs=1) as tsb:

                for q in range(NQ):
                    nc.scalar.activation(th[q], graw[q], AF.Tanh, scale=0.5)
                mcol = ps_t.tile([8, 2], f32, tag="mcol", name="mcol")
                rxA = ps_t.tile([1, 512], f32, tag="rxA", name="rxA")
                rxB = ps_t.tile([1, 512], f32, tag="rxB", name="rxB")
                ab = ps_t.tile([128, 1024], f32, tag="ab", name="ab")

                sy = stats_aps["t"][0:1, 0:512]
                sy2 = stats_aps["t"][0:1, 512:1024]
                gsum = tsb.tile([1, 8], f32, tag="gsum", name="gsum")
                g2 = tsb.tile([1, 8], f32, tag="g2", name="g2")
                nc.vector.tensor_reduce(
                    gsum, sy.rearrange("o (h g e) -> o g h e", h=8, g=8),
                    axis=AX.XY, op=OP.add)
                nc.vector.tensor_reduce(
                    g2, sy2.rearrange("o (h g e) -> o g h e", h=8, g=8),
                    axis=AX.XY, op=OP.add)
                nc.tensor.matmul(mcol[:, 0:1], gsum, one1, start=True, stop=False,
                                 skip_group_check=True)
                nc.tensor.matmul(mcol[:, 1:2], g2, one1, start=False, stop=True,
                                 skip_group_check=True)

                e2t = tsb.tile([8, 1], f32, tag="e2t", name="e2t")
                nm = tsb.tile([8, 1], f32, tag="nm", name="nm")
                veps = tsb.tile([8, 1], f32, tag="veps", name="veps")
                rr = tsb.tile([8, 1], f32, tag="rr", name="rr")
                rsd = tsb.tile([8, 1], f32, tag="rsd", name="rsd")
                ts_(mrst[:, 1:2], mcol[:, 0:1], 1.0 / float(S * H * 8), None, OP.mult)
                ts_(e2t, mcol[:, 1:2], 1.0 / float(S * H * 8), None, OP.mult)
                ts_(nm, mrst[:, 1:2], mrst[:, 1:2], -1.0, OP.mult, OP.mult)
                stt(veps, nm, EPS, e2t, OP.add, OP.add)
                # rsqrt via bit-trick seed + 2 Newton iterations (DVE only,
                # avoids the ACT sqrt table load)
                vi = veps.bitcast(mybir.dt.int32)
                si = rsd.bitcast(mybir.dt.int32)
                ts_(si, vi, 1, None, OP.logical_shift_right)
                ts_(si, si, -1, None, OP.bitwise_xor)
                ts_(si, si, 0x5F3759E0, None, OP.add)
                for _ in range(1):
                    nc.vector.tensor_mul(rr, rsd, rsd)
                    nc.vector.tensor_mul(rr, rr, veps)
                    ts_(rr, rr, -0.5, 1.5, OP.mult, OP.add)
                    nc.vector.tensor_mul(rsd, rsd, rr)
                nc.vector.tensor_copy(mrst[:, 0:1], rsd)

                nc.vector.tensor_copy(mrstb, mrst)
                nc.tensor.matmul(rxA, mrstb[:, 0:1], g8_t[:, :], start=True, stop=True)
                nc.tensor.matmul(rxB, mrstb[:, 1:2], g8_t[:, :], start=True, stop=True)
                arow = tsb.tile([1, 512], bf16, tag="arow", name="arow")
                btmp = tsb.tile([1, 512], f32, tag="btmp", name="btmp")
                brow = tsb.tile([1, 512], bf16, tag="brow", name="brow")
                nc.vector.tensor_mul(arow, ghl_r, rxA)
                nc.vector.tensor_mul(btmp, rxB, arow)
                nc.vector.tensor_sub(brow, bhl_r, btmp)
                nc.tensor.matmul(ab[:, 0:512], ones_r[:], arow,
                                 start=True, stop=True)
                nc.tensor.matmul(ab[:, 512:1024], ones_r[:], brow,
                                 start=True, stop=True)

                ab_sb = tsb.tile([128, 1024], bf16, tag="ab_sb", name="ab_sb")
                nc.vector.tensor_copy(ab_sb[:, 0:512], ab[:, 0:512])
                nc.vector.tensor_copy(ab_sb[:, 512:1024], ab[:, 512:1024])
                # finalize on DVE only (cross-engine DVE/Pool splitting was
                # measured ~2x slower per op: SBUF port contention + sem
                # ping-pong); output DMA round-robins all three queues.
                dmas = (nc.sync.dma_start, nc.scalar.dma_start,
                        nc.gpsimd.dma_start)
                for qb in range(NQ):
                    oq = oqp.tile([128, 512], bf16, tag="oq", name="oq")
                    nc.vector.tensor_mul(oq, yy[qb], ab_sb[:, 0:512])
                    nc.vector.tensor_add(oq, oq, ab_sb[:, 512:1024])
                    stt(oq, th[qb], 1.0, oq, OP.add, OP.mult)
                    dmas[qb % 3](out=out_d[128 * qb:128 * (qb + 1), :], in_=oq)

    nc.finalize()
    return nc


_CACHE = {}


def _get_nc():
    if "nc" not in _CACHE:
        _CACHE["nc"] = build_nc()
    return _CACHE["nc"]


def _host_prep(arrs):
    """Pack weights/biases into device layouts (bf16 x^T chunks etc.)."""
    from ml_dtypes import bfloat16 as bf

    def rpack(w):  # [512, C] -> [128, 4*C] with [p, C*r + c] = w[128r + p, c]
        c = w.shape[1]
        return np.ascontiguousarray(
            w.reshape(4, 128, c).transpose(1, 0, 2).reshape(128, 4 * c)).astype(bf)

    wq, wk, wv = arrs["Wq"], arrs["Wk"], arrs["Wv"]
    wg = np.ascontiguousarray(wq.reshape(DM, 8, 192)[:, :, 128:].reshape(DM, 512))
    bq, bk, bv = arrs["bq"], arrs["bk"], arrs["bv"]
    lam = float(arrs["lam"][0])
    li = float(arrs["lambda_init"][0])
    hl = 0.5 * (1.0 - li)

    bqk = np.zeros((128, 18), np.float32)
    for h in range(8):
        bqk[:, h] = bq[192 * h:192 * h + 128]
        bqk[:, 8 + h] = bk[128 * h:128 * h + 128]
    bqk[:, 16] = -lam
    gb = bq.reshape(8, 192)[:, 128:].reshape(512)
    rows = np.concatenate([
        np.tile(arrs["gamma"], 8) * hl,
        np.tile(arrs["beta"], 8) * hl,
        gb, bv]).astype(np.float32).reshape(1, 2048)
    g8 = np.zeros((8, 512), np.float32)
    cols = np.arange(512)
    g8[(cols % 64) // 8, cols] = 1.0

    rowsb = np.concatenate([gb, bv]).astype(bf).reshape(1, 1024)
    wqq = np.ascontiguousarray(
        wq.reshape(DM, 8, 192)[:, :, 0:128].reshape(DM, 1024))
    shared = {
        "wq": rpack(wqq), "wk": rpack(wk), "wv": rpack(wv), "wg": rpack(wg),
        "bqk": np.ascontiguousarray(bqk), "rows": rows, "rowsb": rowsb,
        "g8": g8.astype(bf),
    }
    in_maps = []
    for i in range(B):
        m = dict(shared)
        for nm, key in (("xq", "query"), ("xk", "key"), ("xv", "values")):
            m[nm] = rpack(np.ascontiguousarray(arrs[key][i].T))
        in_maps.append(m)
    return in_maps


def run(inputs, trace=False, tmpdir=None):
    from concourse.bass_utils import run_bass_kernel_spmd
    nc = _get_nc()
    arrs = {k: np.asarray(v, dtype=np.float32) for k, v in inputs.items()}
    in_maps = _host_prep(arrs)
    res = run_bass_kernel_spmd(nc, in_maps, core_ids=list(range(B)),
                               trace=trace, tmpdir=tmpdir)
    out = np.stack([res.results[i]["out"] for i in range(B)], axis=0)
    return out.astype(np.float32), res


def kernel(**inputs):
    out, _ = run(inputs)
    return out

